# revision 25
# baseline (speedup 1.0000x reference)
"""Tropical (max-plus) linear kernel for Trainium2, 8-core SPMD.

y[b, i] = max_j (W[i, j] + x[b, j]) + bias[i]

Exact algorithm: for each batch row b only columns j with
    x[b, j] >= max_j' x[b, j'] - (Wmax - Wmin)
can attain the max for ANY output i (any winner j* satisfies
W[i,j*] + x[b,j*] >= W[i,jm] + x[b,jm] with jm = argmax x, hence
x[b,j*] >= x[b,jm] - spread).  Taking the max over any superset of
those candidates is bit-exact.  The host selects candidates, packs
them into fixed-length lanes (padded with duplicates of a real
candidate, which cannot change the max), gathers the matching W^T
rows, and the device runs one fused scalar_tensor_tensor
(add + running max) per lane step on the Vector engine.

Raw bass (no TileContext): this toolchain's codegen allows at most one
sync-wait command per instruction, so synchronization is explicit —
standalone wait_ge instructions plus one then_inc per producer.
"""

import sys
import types

import numpy as np

import concourse.bass as bass
from concourse import mybir
from concourse.bass_utils import run_bass_kernel_spmd

# If BASS_TRACE is set, bass_utils imports antenv.axon_hooks, which this
# image may lack. Provide a no-op hook module so tracing degrades
# gracefully instead of crashing.
try:
    import antenv.axon_hooks  # noqa: F401
except ImportError:
    try:
        import antenv

        _hooks = types.ModuleType("antenv.axon_hooks")
        _hooks.get_axon_ntff_profile_hook = lambda: None
        _hooks.set_axon_ntff_profile_hook = lambda h: None
        sys.modules["antenv.axon_hooks"] = _hooks
        antenv.axon_hooks = _hooks
    except ImportError:
        pass

N_CORES = 8

# Filled in by kernel() for the benefit of test harnesses.
LAST_RESULT = None

_NC_CACHE = {}


def _build_nc(A, L, IC):
    """SPMD program: per core, A accumulation units of L fused steps each.

    unit a: acc[:, a*IC:(a+1)*IC] =
        max_k (wg[a][:, k*IC:(k+1)*IC] + xg[:, a*L+k] per-partition)
    """
    nc = bass.Bass()
    wg = nc.declare_dram_parameter(
        "wg", [A, 128, L * IC], mybir.dt.float32, isOutput=False
    )
    xg = nc.declare_dram_parameter("xg", [128, A * L], mybir.dt.float32, isOutput=False)
    y = nc.declare_dram_parameter("y", [128, A * IC], mybir.dt.float32, isOutput=True)

    _build_body(nc, wg, xg, y, A, L, IC)
    return nc


def _build_body(nc, wg, xg, y, A, L, IC):
    from contextlib import ExitStack

    with ExitStack() as ctx:
        block = ctx.enter_context(nc.Block(no_gpsimd_drain=True))
        # A DMA's +16 completion arrives in parts across rings, so a shared
        # counter cannot order multiple in-flight DMAs: one sem per DMA.
        sem_x = ctx.enter_context(nc.semaphore("sem_x"))
        sem_y = ctx.enter_context(nc.semaphore("sem_y"))
        sem_w = [ctx.enter_context(nc.semaphore(f"sem_w{a}")) for a in range(A)]
        # one cumulative DVE-progress sem: value a+1 <=> unit a finished
        sem_d = ctx.enter_context(nc.semaphore("sem_d"))
        wt = ctx.enter_context(
            nc.sbuf_tensor("wt", [128, A * L * IC], mybir.dt.float32)
        )
        xt = ctx.enter_context(nc.sbuf_tensor("xt", [128, A * L], mybir.dt.float32))
        acc = ctx.enter_context(
            nc.sbuf_tensor("acc", [128, A * IC], mybir.dt.float32)
        )

        half = (L * IC) // 2

        @block.sync
        def _(sync):
            # SP ring: first half of every wg unit, then odd y stores.
            for a in range(A):
                base = a * L * IC
                sync.dma_start(
                    out=wt[:, base : base + half], in_=wg[a, :, 0:half]
                ).then_inc(sem_w[a], 16)
            for a in range(1, A, 2):
                sync.wait_ge(sem_d, a + 1)
                sync.dma_start(
                    out=y[:, a * IC : (a + 1) * IC],
                    in_=acc[:, a * IC : (a + 1) * IC],
                ).then_inc(sem_y, 16)
            sync.wait_ge(sem_y, 16 * A)

        @block.scalar
        def _(scalar):
            # ACT ring: xg, second half of every wg unit, even y stores.
            scalar.dma_start(out=xt[:], in_=xg[:]).then_inc(sem_x, 16)
            for a in range(A):
                base = a * L * IC
                scalar.dma_start(
                    out=wt[:, base + half : base + L * IC],
                    in_=wg[a, :, half : L * IC],
                ).then_inc(sem_w[a], 16)
            for a in range(0, A, 2):
                scalar.wait_ge(sem_d, a + 1)
                scalar.dma_start(
                    out=y[:, a * IC : (a + 1) * IC],
                    in_=acc[:, a * IC : (a + 1) * IC],
                ).then_inc(sem_y, 16)
            scalar.wait_ge(sem_y, 16 * A)

        @block.vector
        def _(vector):
            vector.wait_ge(sem_x, 16)
            for a in range(A):
                # two half-DMAs (SP + ACT rings) complete at +16 each
                vector.wait_ge(sem_w[a], 32)
                ac = acc[:, a * IC : (a + 1) * IC]
                for k in range(L):
                    s = a * L + k
                    wk = wt[:, s * IC : (s + 1) * IC]
                    if k == 0:
                        # acc = wg_0 + x_0  (single-src op: 2x fp32 mode)
                        vector.tensor_scalar_add(ac, wk, xt[:, s : s + 1])
                    else:
                        # acc = max(wg_k + x_k, acc)
                        inst = vector.scalar_tensor_tensor(
                            ac,
                            wk,
                            xt[:, s : s + 1],
                            ac,
                            mybir.AluOpType.add,
                            mybir.AluOpType.max,
                        )
                inst.then_inc(sem_d, 1)

    return nc


def _choose_config(S):
    """Pick (IC, nih, A, T, L) minimizing estimated per-core time."""
    best = None
    for IC, nih in ((512, 2), (1024, 1)):
        for A in range(1, 13):
            T = A * N_CORES // nih  # number of 128-lane tiles
            cap = 128 * T
            for L in range(2, 129):
                nl = int(np.ceil(S / L).sum())
                if nl <= cap:
                    # per-partition SBUF bytes: wg + accs + xg
                    sbuf = (A * L * IC + A * IC + A * L) * 4
                    if sbuf > 200 * 1024:
                        break
                    dve_ns = A * L * (IC + 151) / 0.96
                    dma_ns = A * L * IC * 128 * 4 / 358.0
                    cost = max(dve_ns, dma_ns)
                    if best is None or cost < best[0]:
                        best = (cost, IC, nih, A, T, L)
                    break
    _, IC, nih, A, T, L = best
    return IC, nih, A, T, L


def kernel(x, weight, bias):
    global LAST_RESULT
    x = np.ascontiguousarray(np.asarray(x, dtype=np.float32))
    weight = np.ascontiguousarray(np.asarray(weight, dtype=np.float32))
    bias = np.asarray(bias, dtype=np.float32)
    Bn, Jn = x.shape
    In = weight.shape[0]

    # --- candidate selection (exact bound, small fp slack) ---
    m = x.max(axis=1)
    spread = float(weight.max()) - float(weight.min())
    thr = (m.astype(np.float64) - spread - 1e-6).astype(np.float32)
    mask = x >= thr[:, None]
    S = mask.sum(axis=1)

    IC, nih, A, T, L = _choose_config(S)

    # --- lane packing ---
    lanes_bat = []
    lanes_idx = []
    for b in range(Bn):
        idx = np.nonzero(mask[b])[0]
        for s in range(0, len(idx), L):
            chunk = idx[s : s + L]
            if len(chunk) < L:
                chunk = np.concatenate(
                    [chunk, np.full(L - len(chunk), chunk[0], dtype=chunk.dtype)]
                )
            lanes_bat.append(b)
            lanes_idx.append(chunk)
    cap = 128 * T
    n_real = len(lanes_bat)
    assert n_real <= cap
    while len(lanes_bat) < cap:
        lanes_bat.append(0)
        lanes_idx.append(np.zeros(L, dtype=np.int64))
    bat = np.asarray(lanes_bat).reshape(T, 128)
    J = np.asarray(lanes_idx).reshape(T, 128, L)

    # --- gather weights / x values, per core ---
    Wt = np.ascontiguousarray(weight.T)  # [in, out], row j = W[:, j]
    units = [(t, h) for t in range(T) for h in range(nih)]
    gcache = {}
    in_maps = []
    for c in range(N_CORES):
        wg_c = np.empty([A, 128, L, IC], dtype=np.float32)
        xg_c = np.empty([A, 128, L], dtype=np.float32)
        for a, (t, h) in enumerate(units[c * A : (c + 1) * A]):
            if t not in gcache:
                gcache[t] = Wt[J[t]]  # [128, L, out]
            G = gcache[t]
            # [128, L, IC]: row p = concat_k W^T[J[p,k], half]
            wg_c[a] = G[:, :, h * IC : (h + 1) * IC]
            xg_c[a] = x[bat[t][:, None], J[t]]
        # xg laid out [128, A*L] so one DMA loads every per-partition scalar
        xg_flat = np.ascontiguousarray(xg_c.transpose(1, 0, 2).reshape(128, A * L))
        in_maps.append({"wg": wg_c.reshape(A, 128, L * IC), "xg": xg_flat})

    # --- device execution ---
    key = (A, L, IC)
    if key not in _NC_CACHE:
        _NC_CACHE[key] = _build_nc(A, L, IC)
    nc = _NC_CACHE[key]
    res = run_bass_kernel_spmd(nc, in_maps, list(range(N_CORES)))
    LAST_RESULT = res

    # --- host-side combine (duplicate lanes / padding are harmless) ---
    yout = np.full((Bn, In), -np.inf, dtype=np.float32)
    for c in range(N_CORES):
        yc = res.results[c]["y"]  # [128, A * IC]
        for a, (t, h) in enumerate(units[c * A : (c + 1) * A]):
            np.maximum.at(
                yout[:, h * IC : (h + 1) * IC], bat[t], yc[:, a * IC : (a + 1) * IC]
            )
    yout = yout + bias[None, :]
    return yout.astype(np.float32)
